# revision 23
# baseline (speedup 1.0000x reference)
"""Paged GQA decode attention (sparse_attention) on 8 TRN2 NeuronCores.

Sharding: tensor-parallel by KV head (8 heads -> 8 cores). Each core gets its
head's slice of the KV pool as a single interleaved bf16 tensor kv_il[t] =
[k_hi(128) | v_hi(128)], so ONE 512 B dma_gather descriptor per token fetches
both K and V (the cost-model DMA floor: >=512 B descriptors run at full bus
bandwidth; two half-size gathers would cost 2x).

Per core dataflow (specialized at build time on the actual seq_lens split,
identical across cores):
  gather: kv[128, slots, 256] bf16 natural layout (tokens on partitions)
  K^T:    per 128-token slot, PE transpose (identity matmul) -> PSUM, then
          PSUM->SBUF copy alternating between ACT and DVE engines
  QK:     scores^T[s,4] = kT @ qhi + kT @ qlo  (q split hi|lo bf16 on host,
          SCALE prefolded; K single bf16)
  exp:    one ACT Exp per group -> p bf16 in SBUF; pad tails zeroed by
          per-section mask columns (tensor_scalar per-partition multiply)
  PV:     o accum [4, j*128..]: p_slot @ v_slot (single bf16 matmul per slot)
  sums:   ones^T @ p -> per-slot partial sums; final reduction and softmax
          normalization on host (elementwise on the tiny [B,HQ,D] output).
"""

import numpy as np
import ml_dtypes

import concourse.bacc as bacc
import concourse.bass as bass
import concourse.mybir as mybir
import concourse.tile as tile
from concourse.bass_utils import run_bass_kernel_spmd
from concourse.masks import make_identity

B, S, HQ, HKV, D, G = 32, 2048, 32, 8, 128, 4
POOL = B * S
HALF = POOL // 2
SCALE = D ** -0.5
NCORES = 8
GROUPS = 8
RPG = B // GROUPS  # requests per group

BF16 = ml_dtypes.bfloat16

_prog_cache: dict = {}
LAST_RESULT = None  # test.py introspection


def _pad128(n):
    return (n + 127) // 128 * 128


def _layout(meta):
    """meta[g][h][j] = valid token count of request j in half h of group g.

    Returns per group: per-half padded counts/slot tables, per-request slot
    lists + sum ranges + mask column ids, and each group-half's column offset
    into the merged idx tensor.
    """
    info = []
    mask_cols = []  # list of (g, h, j, valid_in_last_slot) -> mask col id
    icol = 0  # running column offset into the merged idx tensor
    for g in range(GROUPS):
        lo_secs, hi_secs = meta[g]
        halves = []
        for h, secs in enumerate((lo_secs, hi_secs)):
            starts, slot_cnt = [], []
            pos = 0
            for j in range(RPG):
                starts.append(pos // 128)
                slot_cnt.append(_pad128(secs[j]) // 128)
                pos += _pad128(secs[j])
            halves.append(dict(n=pos, slots=pos // 128, ioff=icol,
                               starts=starts, slot_cnt=slot_cnt, secs=secs))
            icol += pos // 16
        n_lo_slots = halves[0]["slots"]
        nslots = n_lo_slots + halves[1]["slots"]
        req_slots, req_ranges, req_masks = [], [], []
        for j in range(RPG):
            slots, ranges, masks = [], [], []
            for h in (0, 1):
                hh = halves[h]
                base = 0 if h == 0 else n_lo_slots
                s0, cnt = hh["starts"][j], hh["slot_cnt"][j]
                if cnt:
                    ranges.append((base + s0, cnt))
                    for li in range(cnt):
                        slots.append((h, s0 + li, base + s0 + li))
                    tail = hh["secs"][j] % 128
                    if tail:  # partial last slot -> needs mask col
                        mid = len(mask_cols)
                        mask_cols.append((g, h, j, tail))
                        masks.append((base + s0 + cnt - 1, mid))
            req_slots.append(slots)
            req_ranges.append(ranges)
            req_masks.append(masks)
        info.append(dict(halves=halves, nslots=nslots,
                         req_slots=req_slots, req_ranges=req_ranges,
                         req_masks=req_masks))
    return info, mask_cols, icol


def _build_program(meta):
    info, mask_cols, idx_w = _layout(meta)
    n_mask = max(1, len(mask_cols))
    dt = mybir.dt
    nc = bacc.Bacc(trn_type="TRN2")

    kv_il = nc.dram_tensor("kv_il", [POOL, 256], dt.bfloat16, kind="ExternalInput")
    qhiT = nc.dram_tensor("qhiT", [128, 128], dt.bfloat16, kind="ExternalInput")
    qloT = nc.dram_tensor("qloT", [128, 128], dt.bfloat16, kind="ExternalInput")
    maskc_d = nc.dram_tensor("maskc", [128, n_mask], dt.float32, kind="ExternalInput")
    idx_w = max(1, idx_w)
    idx_d = nc.dram_tensor("idx_all", [128, idx_w], dt.int16, kind="ExternalInput")
    OC = RPG * D  # output cols per group
    o_dram = nc.dram_tensor("o_un", [G, 2 * B * D], dt.float32, kind="ExternalOutput")
    s_dram = nc.dram_tensor("sums", [GROUPS, 512], dt.float32, kind="ExternalOutput")

    def group_chunks(g):
        """Chunks (h, c0, cs): whole halves, except the last group (drain)
        and the first chunk of group 0 (pipeline ramp), which are split."""
        out = []
        for h in (0, 1):
            slots_h = info[g]["halves"][h]["slots"]
            if slots_h == 0:
                continue
            if g == GROUPS - 1:
                c0 = 0
                while c0 < slots_h:
                    cs = min(8, slots_h - c0)
                    out.append((h, c0, cs))
                    c0 += cs
            elif slots_h > 10:
                c1 = (slots_h + 1) // 2
                out.append((h, 0, c1))
                out.append((h, c1, slots_h - c1))
            else:
                out.append((h, 0, slots_h))
        return out

    with tile.TileContext(nc) as tc:
        with (
            tc.tile_pool(name="const", bufs=1) as cpool,
            tc.tile_pool(name="kv", bufs=12) as kvp,
            tc.tile_pool(name="kvl", bufs=8) as kvlp,
            tc.tile_pool(name="kt", bufs=6) as ktp,
            tc.tile_pool(name="pt", bufs=3) as ptp,
            tc.tile_pool(name="stg", bufs=2) as stgp,
            tc.tile_pool(name="ps_tr", bufs=2, space="PSUM") as pstr,
            tc.tile_pool(name="ps_sc", bufs=2, space="PSUM") as pssc,
            tc.tile_pool(name="ps_pv", bufs=2, space="PSUM") as pspv,
        ):
            qhi_t = cpool.tile([128, 128], dt.bfloat16, tag="qhi")
            qlo_t = cpool.tile([128, 128], dt.bfloat16, tag="qlo")
            ones_t = cpool.tile([128, 1], dt.bfloat16, tag="ones")
            ident_t = cpool.tile([128, 128], dt.bfloat16, tag="ident")
            mask_t = cpool.tile([128, n_mask], dt.float32, tag="maskc")
            idx_t = cpool.tile([128, idx_w], dt.int16, tag="idxall")
            nc.vector.memset(ones_t[:], 1.0)
            make_identity(nc, ident_t[:])
            # prefetch every chunk's idx slice up front on SP, in consumption
            # order; q/mask consts slot in right after the first chunk's idx
            first = True
            for g in range(GROUPS):
                for (h, c0, cs) in group_chunks(g):
                    ic0 = info[g]["halves"][h]["ioff"] + 8 * c0
                    nc.sync.dma_start(out=idx_t[:, ic0:ic0 + 8 * cs],
                                      in_=idx_d[:, ic0:ic0 + 8 * cs])
                    if first:
                        nc.sync.dma_start(out=qhi_t[:], in_=qhiT[:])
                        nc.sync.dma_start(out=qlo_t[:], in_=qloT[:])
                        nc.sync.dma_start(out=mask_t[:], in_=maskc_d[:])
                        first = False

            ncopy = 0  # global copy counter for ACT/DVE alternation
            for g in range(GROUPS):
                gi = info[g]
                nslots = gi["nslots"]
                ncols = 4 * nslots
                if nslots == 0:
                    z = stgp.tile([G, 2 * OC], dt.float32, tag="ostg")
                    nc.vector.memset(z[:], 0.0)
                    nc.sync.dma_start(
                        out=o_dram[0:G, 2 * OC * g:2 * OC * (g + 1)],
                        in_=z[:])
                    continue
                n_lo_slots = gi["halves"][0]["slots"]

                def owner(h, loc):
                    hh = gi["halves"][h]
                    return max(jj for jj in range(RPG)
                               if hh["starts"][jj] <= loc)

                mask_by_slot = {}
                for j in range(RPG):
                    for (gslot, mid) in gi["req_masks"][j]:
                        mask_by_slot[gslot] = mid

                pt = ptp.tile([128, ncols], dt.bfloat16, tag="pt")
                # per-half PV banks: request accumulation groups stay
                # sequential within each bank (PSUM start=True marks the
                # whole 2KB bank pending-zero); halves merge on host
                pvh0 = pspv.tile([G, OC], dt.float32, tag="pv0")
                pvh1 = pspv.tile([33, OC], dt.float32, tag="pv1")
                pvh = [pvh0, pvh1[0:G, :]]
                sm = pvh1[32:33, :]
                rh_total = [[gi["halves"][h]["slot_cnt"][j]
                             for j in range(RPG)] for h in (0, 1)]
                rh_done = [[0] * RPG, [0] * RPG]
                for h in (0, 1):
                    for j in range(RPG):
                        if rh_total[h][j] == 0:
                            nc.vector.memset(
                                pvh[h][0:G, 128 * j:128 * j + 128], 0.0)

                kv_tiles = {}  # (h, local slot) -> (tile, col offset)

                def emit_pv(ch):
                    h, c0, cs = ch
                    gbase = 0 if h == 0 else n_lo_slots
                    for i in range(cs):
                        loc = c0 + i
                        gs = gbase + loc
                        j = owner(h, loc)
                        kvt, kc = kv_tiles[(h, loc)]
                        oc = 128 * j
                        nc.tensor.matmul(
                            pvh[h][0:G, oc:oc + 128],
                            pt[:, 4 * gs:4 * gs + 4],
                            kvt[:, kc, 128:256],
                            start=(rh_done[h][j] == 0),
                            stop=(rh_done[h][j] == rh_total[h][j] - 1),
                            skip_group_check=True)
                        rh_done[h][j] += 1

                chunks = group_chunks(g)
                for ci, (h, c0, cs) in enumerate(chunks):
                    hh = gi["halves"][h]
                    gbase = 0 if h == 0 else n_lo_slots
                    n = 128 * cs
                    ic0 = hh["ioff"] + 8 * c0
                    it = idx_t[:, ic0:ic0 + 8 * cs]
                    src = (kv_il[0:HALF, :] if h == 0
                           else kv_il[HALF:POOL, :])
                    pool_ = kvlp if g == GROUPS - 1 else kvp
                    kv = pool_.tile([128, cs, 256], dt.bfloat16, tag="kv")
                    nc.gpsimd.dma_gather(
                        out_ap=kv[:], in_ap=src, idxs_ap=it,
                        num_idxs=n, num_idxs_reg=n, elem_size=256,
                        transpose=False, single_packet=False)
                    for i in range(cs):
                        kv_tiles[(h, c0 + i)] = (kv, i)

                    # K^T: transpose 8 slots/PSUM bank + one batched copy,
                    # then QK for those slots
                    sc = pssc.tile([128, 4 * cs], dt.float32, tag="sc")
                    for i0 in range(0, cs, 8):
                        nb = min(8, cs - i0)
                        tp = pstr.tile([128, 1024], dt.bfloat16, tag="tr")
                        kt = ktp.tile([128, 1024], dt.bfloat16, tag="kt")
                        for i in range(nb):
                            nc.tensor.transpose(
                                tp[:, 128 * i:128 * (i + 1)],
                                kv[:, i0 + i, 0:128], ident_t[:])
                        w = 128 * nb
                        if ncopy & 1:
                            nc.vector.tensor_copy(out=kt[:, 0:w],
                                                  in_=tp[:, 0:w])
                        else:
                            nc.scalar.activation(
                                kt[:, 0:w], tp[:, 0:w],
                                mybir.ActivationFunctionType.Copy)
                        ncopy += 1
                        for i in range(nb):
                            loc = c0 + i0 + i
                            b = RPG * g + owner(h, loc)
                            out = sc[:, 4 * (i0 + i):4 * (i0 + i) + 4]
                            ksl = kt[:, 128 * i:128 * (i + 1)]
                            nc.tensor.matmul(out, ksl,
                                             qhi_t[:, 4 * b:4 * b + 4],
                                             start=True, stop=False,
                                             skip_group_check=True)
                            nc.tensor.matmul(out, ksl,
                                             qlo_t[:, 4 * b:4 * b + 4],
                                             start=False, stop=True,
                                             skip_group_check=True)
                    # exp + tail masks for the chunk
                    pc0 = 4 * (gbase + c0)
                    nc.scalar.activation(pt[:, pc0:pc0 + 4 * cs], sc[:],
                                         mybir.ActivationFunctionType.Exp)
                    for i in range(cs):
                        gs = gbase + c0 + i
                        if gs in mask_by_slot:
                            mid = mask_by_slot[gs]
                            cc = 4 * gs
                            nc.vector.tensor_scalar_mul(
                                out=pt[:, cc:cc + 4],
                                in0=pt[:, cc:cc + 4],
                                scalar1=mask_t[:, mid:mid + 1])
                    # sums for this chunk's columns
                    gs0, gs1 = gbase + c0, gbase + c0 + cs
                    for j in range(RPG):
                        for (s0, cnt) in gi["req_ranges"][j]:
                            a = max(s0, gs0)
                            bnd = min(s0 + cnt, gs1)
                            if a < bnd:
                                nc.tensor.matmul(
                                    sm[0:1, 4 * a:4 * bnd], ones_t[:, 0:1],
                                    pt[:, 4 * a:4 * bnd], start=True,
                                    stop=True, skip_group_check=True)
                    # PV for the previous chunk (one-chunk software pipeline)
                    if ci > 0:
                        emit_pv(chunks[ci - 1])
                emit_pv(chunks[-1])

                ostg = stgp.tile([G, 2 * OC], dt.float32, tag="ostg")
                sstg = stgp.tile([1, 512], dt.float32, tag="sstg")
                nc.vector.tensor_copy(out=ostg[:, 0:OC], in_=pvh[0][:])
                nc.vector.tensor_copy(out=ostg[:, OC:2 * OC], in_=pvh[1][:])
                nc.scalar.activation(sstg[0:1, 0:ncols],
                                     pvh1[32:33, 0:ncols],
                                     mybir.ActivationFunctionType.Copy)
                nc.sync.dma_start(
                    out=o_dram[0:G, 2 * OC * g:2 * OC * (g + 1)],
                    in_=ostg[:])
                nc.sync.dma_start(out=s_dram[g:g + 1, 0:ncols],
                                  in_=sstg[0:1, 0:ncols])

    nc.compile()
    return nc, info, mask_cols


def prepare(inputs):
    q = np.asarray(inputs["q"], np.float32)
    k = np.asarray(inputs["k"], np.float32)
    v = np.asarray(inputs["v"], np.float32)
    k_buffer = np.asarray(inputs["k_buffer"], np.float32)
    v_buffer = np.asarray(inputs["v_buffer"], np.float32)
    req_to_token = np.asarray(inputs["req_to_token"])
    req_pool_indices = np.asarray(inputs["req_pool_indices"])
    seq_lens = np.asarray(inputs["seq_lens"]).astype(np.int64)
    out_cache_loc = np.asarray(inputs["out_cache_loc"]).astype(np.int64)

    # store_kv_cache scatter (tiny: 32 rows) + per-request token lists
    kb = k_buffer.copy()
    vb = v_buffer.copy()
    kb[out_cache_loc] = k.reshape(B, HKV, D)
    vb[out_cache_loc] = v.reshape(B, HKV, D)
    tok = req_to_token[req_pool_indices]

    # smallest group first (fast pipeline fill), next-smallest last (short
    # drain tail), the rest biggest-first in between
    asc = list(np.argsort(seq_lens, kind="stable"))
    head, tail_, mid = asc[RPG:2 * RPG], asc[:RPG], asc[2 * RPG:][::-1]
    order = np.array(head + mid + tail_, dtype=np.int64)

    meta = []
    idx_blocks = []
    for g in range(GROUPS):
        lo_secs, hi_secs = [], []
        for h in (0, 1):
            parts = []
            secs = lo_secs if h == 0 else hi_secs
            for j in range(RPG):
                b = int(order[RPG * g + j])
                t = tok[b, :seq_lens[b]].astype(np.int64)
                tl = t[t < HALF] if h == 0 else t[t >= HALF] - HALF
                secs.append(len(tl))
                arr = np.zeros(_pad128(len(tl)), np.int64)
                arr[:len(tl)] = tl
                parts.append(arr)
            full = np.concatenate(parts)
            if len(full):
                # [16, n/16] wrap, replicated into all 8 GPSIMD-core stripes
                idx_blocks.append(
                    np.tile(full.astype(np.int16).reshape(-1, 16).T, (8, 1)))
        meta.append((tuple(lo_secs), tuple(hi_secs)))
    meta = tuple(meta)
    if idx_blocks:
        idx_all = np.ascontiguousarray(np.concatenate(idx_blocks, axis=1))
    else:
        idx_all = np.zeros((128, 1), np.int16)

    if meta not in _prog_cache:
        _prog_cache[meta] = _build_program(meta)
    nc, info, mask_cols = _prog_cache[meta]

    maskc = np.ones((128, max(1, len(mask_cols))), np.float32)
    for mid, (_, _, _, tail) in enumerate(mask_cols):
        maskc[:, mid] = (np.arange(128) < tail).astype(np.float32)

    in_maps = []
    for c in range(NCORES):
        k_hi = kb[:, c, :].astype(BF16)
        v_hi = vb[:, c, :].astype(BF16)
        qc = (q.reshape(B, HKV, G, D)[order, c] * SCALE).reshape(B * G, D)
        qT = np.ascontiguousarray(qc.T)
        q_hi = qT.astype(BF16)
        q_lo = (qT - q_hi.astype(np.float32)).astype(BF16)
        im = {
            "kv_il": np.ascontiguousarray(np.concatenate([k_hi, v_hi], axis=1)),
            "qhiT": np.ascontiguousarray(q_hi),
            "qloT": np.ascontiguousarray(q_lo),
            "maskc": maskc,
            "idx_all": idx_all,
        }
        in_maps.append(im)
    return nc, info, in_maps, order


def postprocess(results, info, order, cores=None):
    OC = RPG * D
    out = np.zeros((B, HQ, D), np.float32)
    for c in (cores if cores is not None else range(NCORES)):
        o_un = results[c]["o_un"]
        sums = results[c]["sums"]
        for g in range(GROUPS):
            gi = info[g]
            for j in range(RPG):
                b = int(order[RPG * g + j])
                stot = np.zeros(G, np.float64)
                for (s0, cnt) in gi["req_ranges"][j]:
                    seg = sums[g, 4 * s0:4 * (s0 + cnt)].astype(np.float64)
                    stot += seg.reshape(cnt, G).sum(axis=0)
                o0 = o_un[:, 2 * OC * g + 128 * j:
                          2 * OC * g + 128 * (j + 1)]
                o1 = o_un[:, 2 * OC * g + OC + 128 * j:
                          2 * OC * g + OC + 128 * (j + 1)]
                ov = o0 + o1
                with np.errstate(divide="ignore", invalid="ignore"):
                    out[b, c * G:(c + 1) * G, :] = ov / stot[:, None]
    return out.reshape(B, HQ * D).astype(np.float32)


def kernel(**inputs):
    global LAST_RESULT
    nc, info, in_maps, order = prepare(inputs)
    res = run_bass_kernel_spmd(nc, in_maps, core_ids=list(range(NCORES)),
                               trace=False)
    LAST_RESULT = res
    return postprocess(res.results, info, order)
